# revision 13
# baseline (speedup 1.0000x reference)
"""Multi-head attention (B=2, S=2048, D=1024, H=16) on 8 TRN2 NeuronCores.

Sharding: core c handles batch b = c // 4 and heads 4*(c%4) .. 4*(c%4)+3.
Each core computes its 4 heads' Q/K/V projections (column slices of the
weights), head-local attention, and a partial output projection (row slice
of Wo). Host sums the 4 partials per batch. No cross-device collectives.

Structure: the kernel is paced by the ScalarE exp stream (16.8M exps/core
is the largest single-engine cost). Everything else hides inside it:

- Scores for a HEAD PAIR run as two row-packed fp16 matmuls (head dims at
  partitions 0:64 / 64:128 -> tile_position (0,0)/(64,0), concurrent on
  the PE) into one [128, 1024] PSUM tile; ONE exp covers both heads.
- V is produced directly in [keys, dims] layout (lhsT = x^T tile), no PE
  transposes; bias lands via a K=1 ones-outer-product matmul. V tiles
  carry a 64-wide ones block so softmax denominators accumulate as
  replicated rows 64:128 of the PV PSUM tile.
- Slots iterate pair-major (pair0 qc0..3 then pair1 qc0..3, qc = 512-wide
  q chunk). Only K[m0,c0], V tiles 0..3 and Q[m0,qc0] run up front; all
  remaining projection + out-projection work is queued as ~0.5-0.9us
  closures drained into the PE slack of the exp cadence. PSUM accumulator
  tiles are allocated inside the closures so pool-ring reuse order always
  matches execution order.
- ScalarE runs ONLY exps; every DMA trigger lives on Sync/GpSimd queues.
- Output partials are stored fp16 (host accumulates in fp32, adds bo).
"""

import numpy as np

import concourse.bacc as bacc
import concourse.mybir as mybir
import concourse.tile as tile
from concourse.bass_utils import run_bass_kernel_spmd

B, S, D, H = 2, 2048, 1024, 16
HD = D // H            # 64
N_CORES = 8
HPC = H // (N_CORES // B)   # heads per core = 4
HG = HPC * HD               # head-group width = 256

F32 = mybir.dt.float32
F16 = mybir.dt.float16
AF = mybir.ActivationFunctionType
MUL = mybir.AluOpType.mult
P = 128

NDK = D // P     # 8 contraction tiles for projections
NM = HG // P     # 2 row tiles (= head pairs)
NKT = S // P     # 16 key tiles
QA = 512         # attention q chunk
NQA = S // QA    # 4
NOG = D // P     # 8 out-proj row groups
CH = 512         # x-chunk width

_CACHE = {}


def _build():
    nc = bacc.Bacc("TRN2", target_bir_lowering=False, debug=False,
                   num_devices=N_CORES)

    qt_d = nc.dram_tensor("qt", [D, S], F16, kind="ExternalInput")
    kt_d = nc.dram_tensor("kt", [D, S], F16, kind="ExternalInput")
    vt_d = nc.dram_tensor("vt", [D, S], F16, kind="ExternalInput")
    wq_d = nc.dram_tensor("wq", [D, HG], F16, kind="ExternalInput")
    wk_d = nc.dram_tensor("wk", [D, HG], F16, kind="ExternalInput")
    wv_d = nc.dram_tensor("wv", [D, HG], F16, kind="ExternalInput")
    wo_d = nc.dram_tensor("wo", [HG, D], F16, kind="ExternalInput")
    bq_d = nc.dram_tensor("bq", [HG, 1], F32, kind="ExternalInput")
    bk_d = nc.dram_tensor("bk", [HG, 1], F32, kind="ExternalInput")
    bvr_d = nc.dram_tensor("bvr", [1, HG], F16, kind="ExternalInput")
    out_d = nc.dram_tensor("outT", [D, S], F16, kind="ExternalOutput")

    with tile.TileContext(nc) as tc:
        with (
            tc.tile_pool(name="persist", bufs=1) as pp,
            tc.tile_pool(name="s_ps", bufs=2, space="PSUM") as sps,
            tc.tile_pool(name="sm_ps", bufs=2, space="PSUM") as smp,
            tc.tile_pool(name="po_ps", bufs=2, space="PSUM") as pop,
            tc.tile_pool(name="pt_pool", bufs=3) as ptp,
            tc.tile_pool(name="ot_sb", bufs=3) as otp,
            tc.tile_pool(name="sc_sb", bufs=3) as scp,
        ):
            # whole-input staging (written by per-chunk DMAs)
            kx_sb = pp.tile([P, NDK, S], F16, tag="kx")
            vx_sb = pp.tile([P, NDK, S], F16, tag="vx")
            qx_sb = pp.tile([P, NDK, S], F16, tag="qx")
            qt_sb = [pp.tile([P, S], F16, tag=f"qt{m}", name=f"qt_sb{m}")
                     for m in range(NM)]
            kt_sb = [pp.tile([P, S], F16, tag=f"kt{m}", name=f"kt_sb{m}")
                     for m in range(NM)]
            at_sb = [pp.tile([P, S], F16, tag=f"at{m}", name=f"at_sb{m}")
                     for m in range(NM)]
            vb = [pp.tile([P, HPC, 2 * HD], F16, tag=f"vb{f}", name=f"vb{f}")
                  for f in range(NKT)]
            wq_sb = pp.tile([P, NDK, HG], F16, tag="wq")
            wk_sb = pp.tile([P, NDK, HG], F16, tag="wk")
            wv_sb = pp.tile([P, NDK, HG], F16, tag="wv")
            wo_sb = pp.tile([P, NM, D], F16, tag="wo")
            bq_sb = pp.tile([P, NM], F32, tag="bq")
            bk_sb = pp.tile([P, NM], F32, tag="bk")
            bvr_sb = pp.tile([1, HG], F16, tag="bvr")
            ones1 = pp.tile([1, P], F16)

            for f in range(NKT):   # ones blocks (disjoint from V halves)
                nc.gpsimd.memset(vb[f][:, :, HD:2 * HD], 1.0)
            nc.gpsimd.memset(ones1[:], 1.0)

            # ---- input DMAs: ring A = sync, ring B = gpsimd ----
            def xchunk(eng, dst, dram, c):
                sl = slice(c * CH, (c + 1) * CH)
                eng.dma_start(dst[:, :, sl],
                              dram[:, sl].rearrange("(a p) n -> p a n", p=P))

            nc.sync.dma_start(wk_sb[:], wk_d[:].rearrange("(a p) n -> p a n", p=P))
            nc.sync.dma_start(bq_sb[:], bq_d[:].rearrange("(a p) o -> p (a o)", p=P))
            nc.sync.dma_start(bk_sb[:], bk_d[:].rearrange("(a p) o -> p (a o)", p=P))
            nc.gpsimd.dma_start(wv_sb[:], wv_d[:].rearrange("(a p) n -> p a n", p=P))
            nc.gpsimd.dma_start(bvr_sb[:], bvr_d[:])
            xchunk(nc.sync, kx_sb, kt_d, 0)
            xchunk(nc.gpsimd, vx_sb, vt_d, 0)
            nc.gpsimd.dma_start(wq_sb[:], wq_d[:].rearrange("(a p) n -> p a n", p=P))
            xchunk(nc.sync, qx_sb, qt_d, 0)
            xchunk(nc.gpsimd, vx_sb, vt_d, 1)
            xchunk(nc.sync, kx_sb, kt_d, 1)
            xchunk(nc.gpsimd, vx_sb, vt_d, 2)
            xchunk(nc.sync, kx_sb, kt_d, 2)
            xchunk(nc.gpsimd, vx_sb, vt_d, 3)
            xchunk(nc.sync, kx_sb, kt_d, 3)
            xchunk(nc.gpsimd, qx_sb, qt_d, 1)
            nc.sync.dma_start(wo_sb[:], wo_d[:].rearrange("(a p) n -> p a n", p=P))
            xchunk(nc.gpsimd, qx_sb, qt_d, 2)
            xchunk(nc.sync, qx_sb, qt_d, 3)

            # ---- background work units (psum allocated at drain time) ----
            def proj_unit(w_sb, x_sb, b_sb, dst, m, c, pfx):
                csl = slice(c * CH, (c + 1) * CH)
                cols = slice(m * P, (m + 1) * P)
                cell = {}

                def a():
                    ps = smp.tile([P, CH], F32, tag="sm", name=f"ps_{pfx}")
                    cell["ps"] = ps
                    for d in range(4):
                        nc.tensor.matmul(ps[:], w_sb[:, d, cols],
                                         x_sb[:, d, csl],
                                         start=(d == 0), stop=False)

                def b():
                    ps = cell["ps"]
                    for d in range(4, NDK):
                        nc.tensor.matmul(ps[:], w_sb[:, d, cols],
                                         x_sb[:, d, csl],
                                         start=False, stop=(d == NDK - 1))
                    nc.vector.tensor_scalar_add(dst[m][:, csl], ps[:],
                                                b_sb[:, m:m + 1])
                return [a, b]

            def v_unit(f):
                ksl = slice(f * P, (f + 1) * P)
                cell = {}

                def a():
                    ps = smp.tile([P, HG], F32, tag="sm", name="ps_v",
                                  padded_shape=[P, CH])
                    cell["ps"] = ps
                    for d in range(4):
                        nc.tensor.matmul(ps[:], vx_sb[:, d, ksl],
                                         wv_sb[:, d, :],
                                         start=(d == 0), stop=False)

                def b():
                    ps = cell["ps"]
                    for d in range(4, NDK):
                        nc.tensor.matmul(ps[:], vx_sb[:, d, ksl],
                                         wv_sb[:, d, :],
                                         start=False, stop=False)
                    nc.tensor.matmul(ps[:], ones1[:], bvr_sb[:],
                                     start=False, stop=True)
                    nc.vector.tensor_copy(
                        vb[f][:, :, 0:HD],
                        ps[:].rearrange("p (h e) -> p h e", h=HPC))
                return [a, b]

            def op_unit(qc, g):
                qsl = slice(qc * QA, (qc + 1) * QA)
                rows = slice(g * P, (g + 1) * P)

                def a():
                    ps = smp.tile([P, QA], F32, tag="sm", name="ps_o")
                    for m in range(NM):
                        nc.tensor.matmul(ps[:], wo_sb[:, m, rows],
                                         at_sb[m][:, qsl],
                                         start=(m == 0), stop=(m == NM - 1))
                    ot = otp.tile([P, QA], F16, tag="ot", name="ot")
                    nc.vector.tensor_copy(ot[:], ps[:])
                    eng = nc.sync if g % 2 == 0 else nc.gpsimd
                    eng.dma_start(out_d[rows, qsl], ot[:])
                return [a]

            kargs = (wk_sb, kx_sb, bk_sb, kt_sb)
            qargs = (wq_sb, qx_sb, bq_sb, qt_sb)

            # ---- background units keyed by product; require() guarantees a
            # producer is EMITTED before its consumer (Tile deps are
            # program-order RAW edges - a late-emitted producer is no
            # dependency at all). ----
            pending = {}
            fifo = []

            def add(key, cs):
                pending[key] = list(cs)
                fifo.append(key)

            def require(key):
                for u in pending.pop(key, []):
                    u()

            def drain1():
                while fifo and fifo[0] not in pending:
                    fifo.pop(0)
                if fifo:
                    key = fifo[0]
                    cs = pending[key]
                    cs.pop(0)()
                    if not cs:
                        pending.pop(key)
                        fifo.pop(0)

            # pre-phase: K[m0,c0], V tiles 0..3, Q[m0,qc0]
            for u in proj_unit(*kargs, 0, 0, "k"):
                u()
            for f in range(4):
                for u in v_unit(f):
                    u()
            for u in proj_unit(*qargs, 0, 0, "q"):
                u()

            for f in range(4, 6):
                add(("V", f), v_unit(f))
            add(("K", 0, 1), proj_unit(*kargs, 0, 1, "k"))
            for f in range(6, 9):
                add(("V", f), v_unit(f))
            add(("K", 0, 2), proj_unit(*kargs, 0, 2, "k"))
            for f in range(9, 12):
                add(("V", f), v_unit(f))
            add(("K", 0, 3), proj_unit(*kargs, 0, 3, "k"))
            for f in range(12, NKT):
                add(("V", f), v_unit(f))
            add(("Q", 0, 1), proj_unit(*qargs, 0, 1, "q"))
            for c in range(4):
                add(("K", 1, c), proj_unit(*kargs, 1, c, "k"))
            add(("Q", 0, 2), proj_unit(*qargs, 0, 2, "q"))
            add(("Q", 1, 0), proj_unit(*qargs, 1, 0, "q"))
            add(("Q", 0, 3), proj_unit(*qargs, 0, 3, "q"))
            for c in range(1, 4):
                add(("Q", 1, c), proj_unit(*qargs, 1, c, "q"))

            # ---- attention: slots pair-major; per kt: row-packed score
            # pair -> one exp -> pv pair (1-step staggered) ----
            def s_pair(pair, qc, kt):
                kc = slice(kt * P, (kt + 1) * P)
                qj = slice(qc * QA, (qc + 1) * QA)
                ps = sps.tile([P, 2 * QA], F32, tag="s", name="s_ps")
                nc.tensor.matmul(ps[:, 0:QA], kt_sb[pair][0:HD, kc],
                                 qt_sb[pair][0:HD, qj], start=True, stop=True)
                nc.tensor.matmul(ps[:, QA:2 * QA], kt_sb[pair][HD:P, kc],
                                 qt_sb[pair][HD:P, qj], start=True, stop=True)
                pt = ptp.tile([P, 2 * QA], F16, tag="pt", name="pt")
                nc.scalar.activation(pt[:], ps[:], AF.Exp)
                return pt

            def pv_pair(pair, kt, po_e, po_o, pt):
                nc.tensor.matmul(po_e[:], vb[kt][:, 2 * pair, :], pt[:, 0:QA],
                                 start=(kt == 0), stop=(kt == NKT - 1))
                nc.tensor.matmul(po_o[:], vb[kt][:, 2 * pair + 1, :],
                                 pt[:, QA:2 * QA],
                                 start=(kt == 0), stop=(kt == NKT - 1))

            def finish_pair(pair, qc, po_e, po_o):
                qsl = slice(qc * QA, (qc + 1) * QA)
                atq = scp.tile([P, QA], F16, tag="atq", name="atq")
                for j, po in ((0, po_e), (1, po_o)):
                    rs = scp.tile([HD, QA], F32, tag="rs", name="rs")
                    nc.vector.tensor_copy(rs[:], po[HD:P, :])
                    rc = scp.tile([HD, QA], F32, tag="rc", name="rc")
                    nc.vector.reciprocal_approx_fast(rc[:], rs[:])
                    nc.vector.tensor_tensor(atq[j * HD:(j + 1) * HD, :],
                                            po[0:HD, :], rc[:], MUL)
                nc.vector.tensor_copy(at_sb[pair][:, qsl], atq[:])

            for s in range(NM * NQA):
                pair, qc = s // NQA, s % NQA
                require(("Q", pair, qc))
                po_e = pop.tile([P, QA], F32, tag="po", name="po_e")
                po_o = pop.tile([P, QA], F32, tag="po", name="po_o")
                prev_pt = None
                for kt in range(NKT):
                    require(("K", pair, kt // 4))
                    require(("V", kt))
                    pt = s_pair(pair, qc, kt)
                    if prev_pt is not None:
                        pv_pair(pair, kt - 1, po_e, po_o, prev_pt)
                    drain1()
                    prev_pt = pt
                pv_pair(pair, NKT - 1, po_e, po_o, prev_pt)
                finish_pair(pair, qc, po_e, po_o)
                if pair == 1:
                    for g in range(NOG):
                        add(("op", qc, g), op_unit(qc, g))
            while fifo:
                drain1()

    nc.compile()
    return nc


def kernel(query, key, value, Wq, bq, Wk, bk, Wv, bv, Wo, bo):
    if "nc" not in _CACHE:
        _CACHE["nc"] = _build()
    nc = _CACHE["nc"]

    scale = np.float32(1.0 / np.sqrt(HD))
    xt = {}
    for b in range(B):
        xt[("q", b)] = np.ascontiguousarray(query[b].T).astype(np.float16)
        xt[("k", b)] = np.ascontiguousarray(key[b].T).astype(np.float16)
        xt[("v", b)] = np.ascontiguousarray(value[b].T).astype(np.float16)

    in_maps = []
    for c in range(N_CORES):
        b, g = c // (N_CORES // B), c % (N_CORES // B)
        cols = slice(g * HG, (g + 1) * HG)
        in_maps.append({
            "qt": xt[("q", b)],
            "kt": xt[("k", b)],
            "vt": xt[("v", b)],
            "wq": (np.ascontiguousarray(Wq[:, cols]) * scale).astype(np.float16),
            "wk": np.ascontiguousarray(Wk[:, cols]).astype(np.float16),
            "wv": np.ascontiguousarray(Wv[:, cols]).astype(np.float16),
            "wo": np.ascontiguousarray(Wo[cols, :]).astype(np.float16),
            "bq": (bq[cols] * scale).reshape(HG, 1).astype(np.float32),
            "bk": bk[cols].reshape(HG, 1).astype(np.float32),
            "bvr": bv[cols].reshape(1, HG).astype(np.float16),
        })

    global _last_in_maps
    _last_in_maps = in_maps
    res = run_bass_kernel_spmd(nc, in_maps, list(range(N_CORES)))

    out = np.zeros((B, S, D), dtype=np.float32)
    for c in range(N_CORES):
        b = c // (N_CORES // B)
        out[b] += res.results[c]["outT"].T.astype(np.float32)
    out += bo.astype(np.float32)
    return out


# revision 15
# speedup vs baseline: 1.0168x; 1.0168x over previous
"""Multi-head attention (B=2, S=2048, D=1024, H=16) on 8 TRN2 NeuronCores.

Sharding: core c handles batch b = c // 4 and heads 4*(c%4) .. 4*(c%4)+3.
Each core computes its 4 heads' Q/K/V projections (column slices of the
weights), head-local attention, and a partial output projection (row slice
of Wo). Host sums the 4 partials per batch. No cross-device collectives.

Structure: the kernel is paced by the ScalarE exp stream (16.8M exps/core
is the largest single-engine cost). Everything else hides inside it:

- Scores for a HEAD PAIR run as two row-packed fp16 matmuls (head dims at
  partitions 0:64 / 64:128 -> tile_position (0,0)/(64,0), concurrent on
  the PE) into one [128, 1024] PSUM tile; ONE exp covers both heads.
- V is produced directly in [keys, dims] layout (lhsT = x^T tile), no PE
  transposes; bias lands via a K=1 ones-outer-product matmul. V tiles
  carry a 64-wide ones block so softmax denominators accumulate as
  replicated rows 64:128 of the PV PSUM tile.
- Slots iterate pair-major (pair0 qc0..3 then pair1 qc0..3, qc = 512-wide
  q chunk). Only K[m0,c0], V tiles 0..3 and Q[m0,qc0] run up front; all
  remaining projection + out-projection work is queued as ~0.5-0.9us
  closures drained into the PE slack of the exp cadence. PSUM accumulator
  tiles are allocated inside the closures so pool-ring reuse order always
  matches execution order.
- ScalarE runs ONLY exps; every DMA trigger lives on Sync/GpSimd queues.
- Output partials are stored fp16 (host accumulates in fp32, adds bo).
"""

import numpy as np

import concourse.bacc as bacc
import concourse.mybir as mybir
import concourse.tile as tile
from concourse.bass_utils import run_bass_kernel_spmd

B, S, D, H = 2, 2048, 1024, 16
HD = D // H            # 64
N_CORES = 8
HPC = H // (N_CORES // B)   # heads per core = 4
HG = HPC * HD               # head-group width = 256

F32 = mybir.dt.float32
F16 = mybir.dt.float16
AF = mybir.ActivationFunctionType
MUL = mybir.AluOpType.mult
P = 128

NDK = D // P     # 8 contraction tiles for projections
NM = HG // P     # 2 row tiles (= head pairs)
NKT = S // P     # 16 key tiles
QA = 512         # attention q chunk
NQA = S // QA    # 4
NOG = D // P     # 8 out-proj row groups
CH = 512         # x-chunk width

_CACHE = {}


def _build():
    nc = bacc.Bacc("TRN2", target_bir_lowering=False, debug=False,
                   num_devices=N_CORES)

    qt_d = nc.dram_tensor("qt", [D, S], F16, kind="ExternalInput")
    kt_d = nc.dram_tensor("kt", [D, S], F16, kind="ExternalInput")
    vt_d = nc.dram_tensor("vt", [D, S], F16, kind="ExternalInput")
    wq_d = nc.dram_tensor("wq", [D, HG], F16, kind="ExternalInput")
    wk_d = nc.dram_tensor("wk", [D, HG], F16, kind="ExternalInput")
    wv_d = nc.dram_tensor("wv", [D, HG], F16, kind="ExternalInput")
    wo_d = nc.dram_tensor("wo", [HG, D], F16, kind="ExternalInput")
    bq_d = nc.dram_tensor("bq", [HG, 1], F32, kind="ExternalInput")
    bk_d = nc.dram_tensor("bk", [HG, 1], F32, kind="ExternalInput")
    bvr_d = nc.dram_tensor("bvr", [1, HG], F16, kind="ExternalInput")
    out_d = nc.dram_tensor("outT", [D, S], F16, kind="ExternalOutput")

    with tile.TileContext(nc) as tc:
        with (
            tc.tile_pool(name="persist", bufs=1) as pp,
            tc.tile_pool(name="s_ps", bufs=2, space="PSUM") as sps,
            tc.tile_pool(name="sm_ps", bufs=2, space="PSUM") as smp,
            tc.tile_pool(name="po_ps", bufs=2, space="PSUM") as pop,
            tc.tile_pool(name="pt_pool", bufs=3) as ptp,
            tc.tile_pool(name="ot_sb", bufs=3) as otp,
            tc.tile_pool(name="sc_sb", bufs=3) as scp,
        ):
            # whole-input staging (written by per-chunk DMAs)
            kx_sb = pp.tile([P, NDK, S], F16, tag="kx")
            vx_sb = pp.tile([P, NDK, S], F16, tag="vx")
            qx_sb = pp.tile([P, NDK, S], F16, tag="qx")
            qt_sb = [pp.tile([P, S], F16, tag=f"qt{m}", name=f"qt_sb{m}")
                     for m in range(NM)]
            kt_sb = [pp.tile([P, S], F16, tag=f"kt{m}", name=f"kt_sb{m}")
                     for m in range(NM)]
            at_sb = [pp.tile([P, S], F16, tag=f"at{m}", name=f"at_sb{m}")
                     for m in range(NM)]
            vb = [pp.tile([P, HPC, 2 * HD], F16, tag=f"vb{f}", name=f"vb{f}")
                  for f in range(NKT)]
            wq_sb = pp.tile([P, NDK, HG], F16, tag="wq")
            wk_sb = pp.tile([P, NDK, HG], F16, tag="wk")
            wv_sb = pp.tile([P, NDK, HG], F16, tag="wv")
            wo_sb = pp.tile([P, NM, D], F16, tag="wo")
            bq_sb = pp.tile([P, NM], F32, tag="bq")
            bk_sb = pp.tile([P, NM], F32, tag="bk")
            bvr_sb = pp.tile([1, HG], F16, tag="bvr")
            ones1 = pp.tile([1, P], F16)

            # ---- input DMAs: ring A = sync, ring B = gpsimd ----
            def xchunk(eng, dst, dram, c):
                sl = slice(c * CH, (c + 1) * CH)
                eng.dma_start(dst[:, :, sl],
                              dram[:, sl].rearrange("(a p) n -> p a n", p=P))

            nc.sync.dma_start(wk_sb[:], wk_d[:].rearrange("(a p) n -> p a n", p=P))
            nc.gpsimd.dma_start(wv_sb[:], wv_d[:].rearrange("(a p) n -> p a n", p=P))
            nc.gpsimd.dma_start(bvr_sb[:], bvr_d[:])
            nc.gpsimd.dma_start(wq_sb[:], wq_d[:].rearrange("(a p) n -> p a n", p=P))
            xchunk(nc.sync, kx_sb, kt_d, 0)
            xchunk(nc.gpsimd, vx_sb, vt_d, 0)
            nc.sync.dma_start(bq_sb[:], bq_d[:].rearrange("(a p) o -> p (a o)", p=P))
            nc.sync.dma_start(bk_sb[:], bk_d[:].rearrange("(a p) o -> p (a o)", p=P))
            xchunk(nc.gpsimd, qx_sb, qt_d, 0)
            xchunk(nc.sync, kx_sb, kt_d, 1)
            xchunk(nc.gpsimd, vx_sb, vt_d, 1)
            xchunk(nc.sync, kx_sb, kt_d, 2)
            xchunk(nc.gpsimd, vx_sb, vt_d, 2)
            xchunk(nc.sync, kx_sb, kt_d, 3)
            xchunk(nc.gpsimd, vx_sb, vt_d, 3)
            xchunk(nc.sync, qx_sb, qt_d, 1)
            nc.gpsimd.dma_start(wo_sb[:], wo_d[:].rearrange("(a p) n -> p a n", p=P))
            xchunk(nc.sync, qx_sb, qt_d, 2)
            xchunk(nc.gpsimd, qx_sb, qt_d, 3)

            for f in range(NKT):   # ones blocks (disjoint from V halves)
                nc.gpsimd.memset(vb[f][:, :, HD:2 * HD], 1.0)
            nc.gpsimd.memset(ones1[:], 1.0)

            # ---- background work units (psum allocated at drain time) ----
            def proj_unit(w_sb, x_sb, b_sb, dst, m, c, pfx):
                csl = slice(c * CH, (c + 1) * CH)
                cols = slice(m * P, (m + 1) * P)
                cell = {}

                def a():
                    ps = smp.tile([P, CH], F32, tag="sm", name=f"ps_{pfx}")
                    cell["ps"] = ps
                    for d in range(4):
                        nc.tensor.matmul(ps[:], w_sb[:, d, cols],
                                         x_sb[:, d, csl],
                                         start=(d == 0), stop=False)

                def b():
                    ps = cell["ps"]
                    for d in range(4, NDK):
                        nc.tensor.matmul(ps[:], w_sb[:, d, cols],
                                         x_sb[:, d, csl],
                                         start=False, stop=(d == NDK - 1))
                    nc.vector.tensor_scalar_add(dst[m][:, csl], ps[:],
                                                b_sb[:, m:m + 1])
                return [a, b]

            def v_unit(f):
                ksl = slice(f * P, (f + 1) * P)
                cell = {}

                def a():
                    ps = smp.tile([P, HG], F32, tag="sm", name="ps_v",
                                  padded_shape=[P, CH])
                    cell["ps"] = ps
                    for d in range(4):
                        nc.tensor.matmul(ps[:], vx_sb[:, d, ksl],
                                         wv_sb[:, d, :],
                                         start=(d == 0), stop=False)

                def b():
                    ps = cell["ps"]
                    for d in range(4, NDK):
                        nc.tensor.matmul(ps[:], vx_sb[:, d, ksl],
                                         wv_sb[:, d, :],
                                         start=False, stop=False)
                    nc.tensor.matmul(ps[:], ones1[:], bvr_sb[:],
                                     start=False, stop=True)
                    nc.vector.tensor_copy(
                        vb[f][:, :, 0:HD],
                        ps[:].rearrange("p (h e) -> p h e", h=HPC))
                return [a, b]

            def op_unit(qc, g):
                qsl = slice(qc * QA, (qc + 1) * QA)
                rows = slice(g * P, (g + 1) * P)

                def a():
                    ps = smp.tile([P, QA], F32, tag="sm", name="ps_o")
                    for m in range(NM):
                        nc.tensor.matmul(ps[:], wo_sb[:, m, rows],
                                         at_sb[m][:, qsl],
                                         start=(m == 0), stop=(m == NM - 1))
                    ot = otp.tile([P, QA], F16, tag="ot", name="ot")
                    nc.vector.tensor_copy(ot[:], ps[:])
                    eng = nc.sync if g % 2 == 0 else nc.gpsimd
                    eng.dma_start(out_d[rows, qsl], ot[:])
                return [a]

            kargs = (wk_sb, kx_sb, bk_sb, kt_sb)
            qargs = (wq_sb, qx_sb, bq_sb, qt_sb)

            # ---- background units keyed by product; require() guarantees a
            # producer is EMITTED before its consumer (Tile deps are
            # program-order RAW edges - a late-emitted producer is no
            # dependency at all). ----
            pending = {}
            fifo = []

            def add(key, cs):
                pending[key] = list(cs)
                fifo.append(key)

            def require(key):
                for u in pending.pop(key, []):
                    u()

            def drain1():
                while fifo and fifo[0] not in pending:
                    fifo.pop(0)
                if fifo:
                    key = fifo[0]
                    cs = pending[key]
                    cs.pop(0)()
                    if not cs:
                        pending.pop(key)
                        fifo.pop(0)

            # pre-phase: K[m0,c0], V tiles 0..3, Q[m0,qc0]
            for u in proj_unit(*kargs, 0, 0, "k"):
                u()
            for f in range(4):
                for u in v_unit(f):
                    u()
            for u in proj_unit(*qargs, 0, 0, "q"):
                u()

            for f in range(4, 6):
                add(("V", f), v_unit(f))
            add(("K", 0, 1), proj_unit(*kargs, 0, 1, "k"))
            for f in range(6, 9):
                add(("V", f), v_unit(f))
            add(("K", 0, 2), proj_unit(*kargs, 0, 2, "k"))
            for f in range(9, 12):
                add(("V", f), v_unit(f))
            add(("K", 0, 3), proj_unit(*kargs, 0, 3, "k"))
            for f in range(12, NKT):
                add(("V", f), v_unit(f))
            add(("Q", 0, 1), proj_unit(*qargs, 0, 1, "q"))
            for c in range(4):
                add(("K", 1, c), proj_unit(*kargs, 1, c, "k"))
            add(("Q", 0, 2), proj_unit(*qargs, 0, 2, "q"))
            add(("Q", 1, 0), proj_unit(*qargs, 1, 0, "q"))
            add(("Q", 0, 3), proj_unit(*qargs, 0, 3, "q"))
            for c in range(1, 4):
                add(("Q", 1, c), proj_unit(*qargs, 1, c, "q"))

            # ---- attention: slots pair-major; per kt: row-packed score
            # pair -> one exp -> pv pair (1-step staggered) ----
            def s_pair(pair, qc, kt):
                kc = slice(kt * P, (kt + 1) * P)
                qj = slice(qc * QA, (qc + 1) * QA)
                ps = sps.tile([P, 2 * QA], F32, tag="s", name="s_ps")
                nc.tensor.matmul(ps[:, 0:QA], kt_sb[pair][0:HD, kc],
                                 qt_sb[pair][0:HD, qj], start=True, stop=True)
                nc.tensor.matmul(ps[:, QA:2 * QA], kt_sb[pair][HD:P, kc],
                                 qt_sb[pair][HD:P, qj], start=True, stop=True)
                pt = ptp.tile([P, 2 * QA], F16, tag="pt", name="pt")
                nc.scalar.activation(pt[:], ps[:], AF.Exp)
                return pt

            def pv_pair(pair, kt, po_e, po_o, pt):
                nc.tensor.matmul(po_e[:], vb[kt][:, 2 * pair, :], pt[:, 0:QA],
                                 start=(kt == 0), stop=(kt == NKT - 1))
                nc.tensor.matmul(po_o[:], vb[kt][:, 2 * pair + 1, :],
                                 pt[:, QA:2 * QA],
                                 start=(kt == 0), stop=(kt == NKT - 1))

            def finish_pair(pair, qc, po_e, po_o):
                qsl = slice(qc * QA, (qc + 1) * QA)
                atq = scp.tile([P, QA], F16, tag="atq", name="atq")
                for j, po in ((0, po_e), (1, po_o)):
                    rs = scp.tile([HD, QA], F32, tag="rs", name="rs")
                    nc.vector.tensor_copy(rs[:], po[HD:P, :])
                    rc = scp.tile([HD, QA], F32, tag="rc", name="rc")
                    nc.vector.reciprocal_approx_fast(rc[:], rs[:])
                    nc.vector.tensor_tensor(atq[j * HD:(j + 1) * HD, :],
                                            po[0:HD, :], rc[:], MUL)
                nc.vector.tensor_copy(at_sb[pair][:, qsl], atq[:])

            for s in range(NM * NQA):
                pair, qc = s // NQA, s % NQA
                require(("Q", pair, qc))
                po_e = pop.tile([P, QA], F32, tag="po", name="po_e")
                po_o = pop.tile([P, QA], F32, tag="po", name="po_o")
                prev_pt = None
                for kt in range(NKT):
                    require(("K", pair, kt // 4))
                    require(("V", kt))
                    pt = s_pair(pair, qc, kt)
                    if prev_pt is not None:
                        pv_pair(pair, kt - 1, po_e, po_o, prev_pt)
                    drain1()
                    prev_pt = pt
                pv_pair(pair, NKT - 1, po_e, po_o, prev_pt)
                finish_pair(pair, qc, po_e, po_o)
                if pair == 1:
                    for g in range(NOG):
                        add(("op", qc, g), op_unit(qc, g))
            while fifo:
                drain1()

    nc.compile()
    return nc


def kernel(query, key, value, Wq, bq, Wk, bk, Wv, bv, Wo, bo):
    if "nc" not in _CACHE:
        _CACHE["nc"] = _build()
    nc = _CACHE["nc"]

    scale = np.float32(1.0 / np.sqrt(HD))
    xt = {}
    for b in range(B):
        xt[("q", b)] = np.ascontiguousarray(query[b].T).astype(np.float16)
        xt[("k", b)] = np.ascontiguousarray(key[b].T).astype(np.float16)
        xt[("v", b)] = np.ascontiguousarray(value[b].T).astype(np.float16)

    in_maps = []
    for c in range(N_CORES):
        b, g = c // (N_CORES // B), c % (N_CORES // B)
        cols = slice(g * HG, (g + 1) * HG)
        in_maps.append({
            "qt": xt[("q", b)],
            "kt": xt[("k", b)],
            "vt": xt[("v", b)],
            "wq": (np.ascontiguousarray(Wq[:, cols]) * scale).astype(np.float16),
            "wk": np.ascontiguousarray(Wk[:, cols]).astype(np.float16),
            "wv": np.ascontiguousarray(Wv[:, cols]).astype(np.float16),
            "wo": np.ascontiguousarray(Wo[cols, :]).astype(np.float16),
            "bq": (bq[cols] * scale).reshape(HG, 1).astype(np.float32),
            "bk": bk[cols].reshape(HG, 1).astype(np.float32),
            "bvr": bv[cols].reshape(1, HG).astype(np.float16),
        })

    global _last_in_maps
    _last_in_maps = in_maps
    res = run_bass_kernel_spmd(nc, in_maps, list(range(N_CORES)))

    out = np.zeros((B, S, D), dtype=np.float32)
    for c in range(N_CORES):
        b = c // (N_CORES // B)
        out[b] += res.results[c]["outT"].T.astype(np.float32)
    out += bo.astype(np.float32)
    return out
